# revision 1
# baseline (speedup 1.0000x reference)
"""Trainium2 Bass kernel for the GCA sparse-attention module.

Math (per batch b):
    a  = emb_a[word_seq] @ lin_w + lin_b                    # [W, H]
    u  = hidden @ a.T / sqrt(H)                             # [L, W]
    e  = exp(u) * (label > 0)                               # [L, W]
    p  = e / (sum_w e + 1e-10)
    o  = sum_w p * emb_c[label]                             # [L, H]

Key restructure: labels take only 6 values, so
    o[l] = (qe[l] / (s[l] + eps)) @ emb_c,   qe[l, n] = sum_w e[l, w] * [label[l, w] == n]
which avoids materializing the [L, W, H] gather entirely.

Sharding: 8 cores = (batch b, L-half) pairs; each core is fully independent
(emb_a table replicated; the kernel gathers only its 256 rows via indirect DMA).
"""

import numpy as np

import concourse.bass as bass
import concourse.mybir as mybir
import concourse.tile as tile
from concourse import bacc
from concourse import bass_utils
from concourse.masks import make_identity

# Problem shapes (hardcoded per contract).
B, L, W = 4, 512, 256
VOCAB, E, H = 30000, 300, 768
NL = 6
P = 128
NCORES = 8
LC = L * B // NCORES        # 256 l-rows per core
WT = W // P                 # 2 w-tiles
LT = LC // P                # 2 l-tiles
HT = H // P                 # 6 h-tiles
EC = [128, 128, 44]         # E=300 split into k-chunks
TEMPER = float(H) ** 0.5

F32 = mybir.dt.float32
I32 = mybir.dt.int32

TRACE = False  # test.py flips this for profiled runs

_CACHE = {}


def _build():
    """Build + compile the per-core Bass program (identical on all cores)."""
    nc = bacc.Bacc("TRN2", debug=False, num_devices=1)

    emb_a = nc.dram_tensor("emb_a", [VOCAB, E], F32, kind="ExternalInput").ap()
    widx = nc.dram_tensor("widx", [P, WT], I32, kind="ExternalInput").ap()
    hT_d = nc.dram_tensor("hT", [H, LC], F32, kind="ExternalInput").ap()
    lab_d = nc.dram_tensor("label", [LC, W], I32, kind="ExternalInput").ap()
    lw_d = nc.dram_tensor("lin_w", [E, H], F32, kind="ExternalInput").ap()
    lb_d = nc.dram_tensor("lin_b", [P, HT], F32, kind="ExternalInput").ap()
    ec_d = nc.dram_tensor("emb_c", [NL, H], F32, kind="ExternalInput").ap()
    o_d = nc.dram_tensor("o", [LC, H], F32, kind="ExternalOutput").ap()

    with tile.TileContext(nc) as tc:
        with (
            tc.tile_pool(name="cst", bufs=1) as cst,
            tc.tile_pool(name="sb", bufs=1) as sb,
            tc.tile_pool(name="wrk", bufs=3) as wrk,
            tc.tile_pool(name="ps", bufs=2, space="PSUM") as ps,
            tc.tile_pool(name="pst", bufs=2, space="PSUM") as pst,
        ):
            ident = cst.tile([P, P], F32, name="ident")
            make_identity(nc, ident[:])

            # ---- load indices / weights / bias / emb_c ----
            wt = cst.tile([P, WT], I32, name="wt")
            nc.sync.dma_start(out=wt[:], in_=widx)

            lb = cst.tile([P, HT], F32, name="lb")
            nc.sync.dma_start(out=lb[:], in_=lb_d)

            ec = cst.tile([NL, H], F32, name="ec")
            nc.sync.dma_start(out=ec[:], in_=ec_d)

            lw = []
            for k in range(3):
                t = sb.tile([P, H], F32, name=f"lw{k}", tag=f"lw{k}")
                nc.sync.dma_start(out=t[: EC[k], :], in_=lw_d[k * P : k * P + EC[k], :])
                lw.append(t)

            # ---- gather emb_a rows: aw[j] = emb_a[widx[:, j]]  [128, 300] ----
            aw = []
            for j in range(WT):
                t = sb.tile([P, E], F32, name=f"aw{j}", tag=f"aw{j}")
                nc.gpsimd.indirect_dma_start(
                    out=t[:],
                    out_offset=None,
                    in_=emb_a,
                    in_offset=bass.IndirectOffsetOnAxis(ap=wt[:, j : j + 1], axis=0),
                )
                aw.append(t)

            # ---- transpose gathered rows: awT[k] = aw.T chunk  [kc, 256] ----
            awT = []
            for k in range(3):
                t = sb.tile([P, WT * P], F32, name=f"awT{k}", tag=f"awT{k}")
                awT.append(t)
            for j in range(WT):
                for k in range(3):
                    kc = EC[k]
                    pt = pst.tile([P, P], F32, name="pt", tag="pt")
                    nc.tensor.transpose(
                        out=pt[:kc, :], in_=aw[j][:, k * P : k * P + kc], identity=ident[:]
                    )
                    nc.scalar.copy(out=awT[k][:kc, j * P : (j + 1) * P], in_=pt[:kc, :])

            # ---- aT[m] = (lin_w.T @ awT)[m-tile] + lin_b  [128, 256] ----
            aT = []
            for m in range(HT):
                pa = ps.tile([P, WT * P], F32, name="pa", tag="pa")
                for k in range(3):
                    kc = EC[k]
                    nc.tensor.matmul(
                        out=pa[:],
                        lhsT=lw[k][:kc, m * P : (m + 1) * P],
                        rhs=awT[k][:kc, :],
                        start=(k == 0),
                        stop=(k == 2),
                    )
                t = sb.tile([P, WT * P], F32, name=f"aT{m}", tag=f"aT{m}")
                # aT = pa + lin_b[m-tile]  (Identity LUT does exact bias-add)
                nc.scalar.activation(
                    out=t[:], in_=pa[:],
                    func=mybir.ActivationFunctionType.Identity,
                    bias=lb[:, m : m + 1], scale=1.0,
                )
                aT.append(t)

            # ---- hidden^T tiles ----
            hm = []
            for m in range(HT):
                t = sb.tile([P, LC], F32, name=f"hm{m}", tag=f"hm{m}")
                nc.sync.dma_start(out=t[:], in_=hT_d[m * P : (m + 1) * P, :])
                hm.append(t)

            # ---- labels ----
            labf = []
            for i in range(LT):
                ti = sb.tile([P, W], I32, name=f"lab{i}", tag=f"lab{i}")
                nc.sync.dma_start(out=ti[:], in_=lab_d[i * P : (i + 1) * P, :])
                tf = sb.tile([P, W], F32, name=f"labf{i}", tag=f"labf{i}")
                nc.vector.tensor_copy(out=tf[:], in_=ti[:])
                labf.append(tf)

            # ---- per l-tile: u, e, masked label sums, normalize, output ----
            for i in range(LT):
                pu = ps.tile([P, W], F32, name="pu", tag="pu")
                for m in range(HT):
                    nc.tensor.matmul(
                        out=pu[:],
                        lhsT=hm[m][:, i * P : (i + 1) * P],
                        rhs=aT[m][:],
                        start=(m == 0),
                        stop=(m == HT - 1),
                    )
                e = sb.tile([P, W], F32, name=f"e{i}", tag=f"e{i}")
                nc.scalar.activation(
                    out=e[:], in_=pu[:],
                    func=mybir.ActivationFunctionType.Exp,
                    scale=1.0 / TEMPER,
                )

                # qe[:, n] = sum_w e * (label == n), n = 1..5 (col 0 stays 0)
                qe = sb.tile([P, NL], F32, name=f"qe{i}", tag=f"qe{i}")
                nc.vector.memset(qe[:, 0:1], 0.0)
                for n in range(1, NL):
                    mask = wrk.tile([P, W], F32, name="mask", tag="mask")
                    nc.vector.tensor_scalar(
                        out=mask[:], in0=labf[i][:],
                        scalar1=float(n), scalar2=None,
                        op0=mybir.AluOpType.is_equal,
                    )
                    nc.vector.tensor_mul(out=mask[:], in0=mask[:], in1=e[:])
                    nc.vector.tensor_reduce(
                        out=qe[:, n : n + 1], in_=mask[:],
                        axis=mybir.AxisListType.X, op=mybir.AluOpType.add,
                    )

                # r = 1 / (sum_n qe + eps)
                s = sb.tile([P, 1], F32, name=f"s{i}", tag=f"s{i}")
                nc.vector.tensor_reduce(
                    out=s[:], in_=qe[:], axis=mybir.AxisListType.X,
                    op=mybir.AluOpType.add,
                )
                nc.vector.tensor_scalar_add(out=s[:], in0=s[:], scalar1=1e-10)
                r = sb.tile([P, 1], F32, name=f"r{i}", tag=f"r{i}")
                nc.vector.reciprocal(out=r[:], in_=s[:])

                # qT = qe.T  [6, 128]
                pq = pst.tile([P, P], F32, name="pq", tag="pt")
                nc.tensor.transpose(out=pq[:NL, :], in_=qe[:], identity=ident[:])
                qT = sb.tile([NL, P], F32, name=f"qT{i}", tag=f"qT{i}")
                nc.scalar.copy(out=qT[:], in_=pq[:NL, :])

                # o = (qT.T @ emb_c) * r   [128, 768]
                o = sb.tile([P, H], F32, name=f"o{i}", tag=f"o{i}")
                for c in range(2):
                    po = ps.tile([P, H // 2], F32, name="po", tag="po")
                    nc.tensor.matmul(
                        out=po[:],
                        lhsT=qT[:],
                        rhs=ec[:, c * (H // 2) : (c + 1) * (H // 2)],
                        start=True,
                        stop=True,
                    )
                    nc.scalar.activation(
                        out=o[:, c * (H // 2) : (c + 1) * (H // 2)], in_=po[:],
                        func=mybir.ActivationFunctionType.Copy,
                        bias=0.0, scale=r[:, 0:1],
                    )
                nc.sync.dma_start(out=o_d[i * P : (i + 1) * P, :], in_=o[:])

    nc.compile()
    return nc


def _get_nc():
    if "nc" not in _CACHE:
        _CACHE["nc"] = _build()
    return _CACHE["nc"]


def kernel(**inputs):
    ws = np.asarray(inputs["word_seq"]).astype(np.int32)          # [B, W]
    hs = np.ascontiguousarray(np.asarray(inputs["hidden_state"], dtype=np.float32))
    lvm = np.asarray(inputs["label_value_matrix"]).astype(np.int32)
    ea = np.ascontiguousarray(np.asarray(inputs["emb_a"], dtype=np.float32))
    lw = np.ascontiguousarray(np.asarray(inputs["lin_w"], dtype=np.float32))
    lb = np.asarray(inputs["lin_b"], dtype=np.float32)
    ec = np.ascontiguousarray(np.asarray(inputs["emb_c"], dtype=np.float32))

    nc = _get_nc()

    lb_t = np.ascontiguousarray(lb.reshape(HT, P).T)
    in_maps = []
    for c in range(NCORES):
        b, half = divmod(c, 2)
        lsl = slice(half * LC, (half + 1) * LC)
        in_maps.append({
            "emb_a": ea,
            "widx": np.ascontiguousarray(ws[b].reshape(WT, P).T),
            "hT": np.ascontiguousarray(hs[b, lsl].T),
            "label": np.ascontiguousarray(lvm[b, lsl]),
            "lin_w": lw,
            "lin_b": lb_t,
            "emb_c": ec,
        })

    res = bass_utils.run_bass_kernel_spmd(
        nc, in_maps, core_ids=list(range(NCORES)), trace=TRACE
    )
    _CACHE["last_result"] = res

    out = np.empty((B, L, H), np.float32)
    for c in range(NCORES):
        b, half = divmod(c, 2)
        out[b, half * LC : (half + 1) * LC] = res.results[c]["o"]
    return out



# revision 16
# speedup vs baseline: 1.2848x; 1.2848x over previous
"""Trainium2 Bass kernel for the GCA sparse-attention module.

Math (per batch b):
    a  = emb_a[word_seq] @ lin_w + lin_b                    # [W, H]
    u  = hidden @ a.T / sqrt(H)                             # [L, W]
    e  = exp(u) * (label > 0)                               # [L, W]
    p  = e / (sum_w e + 1e-10)
    o  = sum_w p * emb_c[label]                             # [L, H]

Restructures vs the straightforward version:
  * labels take only 6 values, so o[l] = (qe[l]/denom[l]) @ emb_c with
    qe[l, n] = sum_w e[l, w] * [label[l, w] == n]  (label 0 dropped: K=5).
  * u is contracted over E, not H:  u = (lin_w' @ h.T).T @ e'.T with the
    bias folded in as a 301st embedding column of ones.
  * everything bf16 on device (1 PE cycle/row vs 4 for fp32; half the DMA),
    fp32 PSUM accumulation; output returned bf16 and upcast on host.
  * gathered-row transpose via the DMA XBAR (no PE transposes / PSUM copies).
  * label one-hot masks precomputed early (Pool + DVE) from the label plane;
    the e-weighted bucket sums use DVE's fused tensor_tensor_reduce.

Sharding: 8 cores = (batch b, L-half) pairs; each core fully independent
(emb_a table replicated; each core gathers its 256 rows via indirect DMA).
"""

import numpy as np
import ml_dtypes

import concourse.bass as bass
import concourse.mybir as mybir
import concourse.tile as tile
from concourse import bacc
from concourse import bass_utils
from concourse.masks import make_identity

# Problem shapes (hardcoded per contract).
B, L, W = 4, 512, 256
VOCAB, E, H = 30000, 300, 768
NL = 6
P = 128
NCORES = 8
LC = L * B // NCORES        # 256 l-rows per core
WT = W // P                 # 2 w-tiles
LT = LC // P                # 2 l-tiles
KT = H // P                 # 6 h-chunks (contraction for hprojT)
EP = E + 1                  # 301: embedding dim + folded bias column
EC = [P, P, EP - 2 * P]     # 301 split into e'-chunks [128, 128, 45]
TEMPER = float(H) ** 0.5

F32 = mybir.dt.float32
BF16 = mybir.dt.bfloat16
I32 = mybir.dt.int32
BF_NP = ml_dtypes.bfloat16

USE_XBAR = True   # DMA XBAR transpose for gathered rows (vs PE transpose)
USE_TTR = False   # fused DVE tensor_tensor_reduce crashes the device (NRT
                  # exec-unit unrecoverable) in this runtime; keep split ops
TRACE = False     # test.py flips this for profiled runs

_CACHE = {}


def _build():
    """Build + compile the per-core Bass program (identical on all cores)."""
    nc = bacc.Bacc("TRN2", debug=False, num_devices=1)

    ea_d = nc.dram_tensor("emb_a", [VOCAB, E], BF16, kind="ExternalInput").ap()
    widx_d = nc.dram_tensor("widx", [P, WT], I32, kind="ExternalInput").ap()
    hm_d = nc.dram_tensor("hm", [P, KT * LC], BF16, kind="ExternalInput").ap()
    lwt_d = nc.dram_tensor("lwt", [P, KT * EP], BF16, kind="ExternalInput").ap()
    lab_d = nc.dram_tensor("label", [P, LT * W], BF16, kind="ExternalInput").ap()
    ec_d = nc.dram_tensor("emb_c", [NL - 1, H], BF16, kind="ExternalInput").ap()
    o_d = nc.dram_tensor("o", [P, LT * H], BF16, kind="ExternalOutput").ap()

    with tile.TileContext(nc) as tc:
        with (
            tc.tile_pool(name="cst", bufs=1) as cst,
            tc.tile_pool(name="sb", bufs=1) as sb,
            tc.tile_pool(name="ps", bufs=3, space="PSUM") as ps,
            tc.tile_pool(name="pst", bufs=2, space="PSUM") as pst,
            tc.tile_pool(name="pso", bufs=2, space="PSUM") as pso,
        ):
            # ---- input DMAs (two dispatch engines so transfers overlap) ----
            wt = cst.tile([P, WT], I32, name="wt")
            nc.sync.dma_start(out=wt[:], in_=widx_d)

            labf = cst.tile([P, LT * W], BF16, name="labf")
            nc.scalar.dma_start(out=labf[:], in_=lab_d)

            hm = cst.tile([P, KT * LC], BF16, name="hm")
            nc.sync.dma_start(out=hm[:], in_=hm_d)

            lwt = cst.tile([P, KT * EP], BF16, name="lwt")
            nc.scalar.dma_start(out=lwt[:], in_=lwt_d)

            ec = cst.tile([NL - 1, H], BF16, name="ec")
            nc.sync.dma_start(out=ec[:], in_=ec_d)

            # ---- gather emb_a rows: aw[j] = emb_a[widx[:, j]]  [128, 300] ----
            # aw is padded to 3*128 cols; col 300 = ones (folded bias), so the
            # transposed chunks carry the ones row at awT2 row 44.
            aw = []
            for j in range(WT):
                t = sb.tile([P, 3 * P], BF16, name=f"aw{j}", tag=f"aw{j}")
                nc.gpsimd.indirect_dma_start(
                    out=t[:, :E],
                    out_offset=None,
                    in_=ea_d,
                    in_offset=bass.IndirectOffsetOnAxis(ap=wt[:, j : j + 1], axis=0),
                )
                nc.gpsimd.memset(t[:, E : E + 1], 1.0)
                nc.gpsimd.memset(t[:, E + 1 :], 0.0)
                aw.append(t)

            # ---- label one-hot masks (early; labels-only dependency) ----
            # Pool: n=1..3, DVE: n=4..5; all [128, 512] covering both l-tiles.
            masks = {}
            for n in range(1, NL):
                m = sb.tile([P, LT * W], BF16, name=f"m{n}", tag=f"m{n}")
                eng = nc.gpsimd if n <= 3 else nc.vector
                eng.tensor_scalar(
                    out=m[:], in0=labf[:],
                    scalar1=float(n), scalar2=None,
                    op0=mybir.AluOpType.is_equal,
                )
                masks[n] = m

            # ---- awT[k] = gathered rows transposed  [<=128, 256] ----
            awT = []
            for k in range(3):
                t = sb.tile([P, WT * P], BF16, name=f"awT{k}", tag=f"awT{k}")
                awT.append(t)
            if USE_XBAR:
                for j in range(WT):
                    for k in range(3):
                        nc.sync.dma_start_transpose(
                            out=awT[k][:, j * P : (j + 1) * P],
                            in_=aw[j][:, k * P : (k + 1) * P],
                        )
            else:
                ident = cst.tile([P, P], BF16, name="ident")
                make_identity(nc, ident[:])
                cp_eng = [nc.vector, nc.scalar, nc.vector,
                          nc.scalar, nc.vector, nc.scalar]
                for j in range(WT):
                    for k in range(3):
                        pt = pst.tile([P, P], BF16, name="pt", tag="pt")
                        nc.tensor.transpose(
                            out=pt[:], in_=aw[j][:, k * P : (k + 1) * P],
                            identity=ident[:],
                        )
                        e_ = cp_eng[j * 3 + k]
                        if e_ is nc.scalar:
                            e_.copy(out=awT[k][:, j * P : (j + 1) * P], in_=pt[:])
                        else:
                            e_.tensor_copy(out=awT[k][:, j * P : (j + 1) * P],
                                           in_=pt[:])

            # ---- hprojT[t] = (lin_w' @ h.T)[e-tile t]  [ec_t, 256] ----
            # lwt layout: [128, k, 301] (k = h-chunk); hm layout: [128, k, 256]
            hp = []
            for t in range(3):
                ew = EC[t]
                toff = t * P
                pp = ps.tile([P, LC], F32, name="pp", tag="pp")
                for k in range(KT):
                    nc.tensor.matmul(
                        out=pp[:ew, :],
                        lhsT=lwt[:, k * EP + toff : k * EP + toff + ew],
                        rhs=hm[:, k * LC : (k + 1) * LC],
                        start=(k == 0),
                        stop=(k == KT - 1),
                    )
                ht = sb.tile([ew, LC], BF16, name=f"hp{t}", tag=f"hp{t}")
                nc.scalar.copy(out=ht[:], in_=pp[:ew, :])
                hp.append(ht)

            # ---- per l-tile: scores, exp, bucket sums, normalize, output ----
            e_t = []
            for i in range(LT):
                pu = ps.tile([P, W], F32, name="pu", tag="pp")
                for k in range(3):
                    kc = EC[k]
                    nc.tensor.matmul(
                        out=pu[:],
                        lhsT=hp[k][:kc, i * P : (i + 1) * P],
                        rhs=awT[k][:kc, :],
                        start=(k == 0),
                        stop=(k == 2),
                    )
                e = sb.tile([P, W], BF16, name=f"e{i}", tag=f"e{i}")
                nc.scalar.activation(
                    out=e[:], in_=pu[:],
                    func=mybir.ActivationFunctionType.Exp,
                    scale=1.0 / TEMPER,
                )
                e_t.append(e)

            qs_t = []
            for i in range(LT):
                e = e_t[i]
                qe = sb.tile([P, NL - 1], F32, name=f"qe{i}", tag=f"qe{i}")
                for n in range(1, NL):
                    scr = sb.tile([P, W], BF16, name="scr", tag=f"scr{i}")
                    if USE_TTR:
                        nc.vector.tensor_tensor_reduce(
                            out=scr[:],
                            in0=masks[n][:, i * W : (i + 1) * W], in1=e[:],
                            scale=1.0, scalar=0.0,
                            op0=mybir.AluOpType.mult,
                            op1=mybir.AluOpType.add,
                            accum_out=qe[:, n - 1 : n],
                        )
                    else:
                        nc.vector.tensor_tensor(
                            out=scr[:],
                            in0=masks[n][:, i * W : (i + 1) * W], in1=e[:],
                            op=mybir.AluOpType.mult,
                        )
                        nc.vector.tensor_reduce(
                            out=qe[:, n - 1 : n], in_=scr[:],
                            axis=mybir.AxisListType.X, op=mybir.AluOpType.add,
                        )
                # r = 1/sum_n qe ; qs = qe * r  (bf16)
                s = sb.tile([P, 1], F32, name=f"s{i}", tag=f"s{i}")
                nc.vector.tensor_reduce(
                    out=s[:], in_=qe[:], axis=mybir.AxisListType.X,
                    op=mybir.AluOpType.add,
                )
                r = sb.tile([P, 1], F32, name=f"r{i}", tag=f"r{i}")
                nc.vector.reciprocal(out=r[:], in_=s[:])
                qs = sb.tile([P, NL - 1], BF16, name=f"qs{i}", tag=f"qs{i}")
                nc.vector.tensor_scalar(
                    out=qs[:], in0=qe[:],
                    scalar1=r[:, 0:1], scalar2=None,
                    op0=mybir.AluOpType.mult,
                )
                qs_t.append(qs)

            identq = cst.tile([P, P], BF16, name="identq")
            make_identity(nc, identq[:])
            for i in range(LT):
                # qsT = qs.T  [5, 128]
                pq = pst.tile([P, P], BF16, name="pq", tag="pt")
                nc.tensor.transpose(out=pq[: NL - 1, :], in_=qs_t[i][:],
                                    identity=identq[:])
                qsT = sb.tile([NL - 1, P], BF16, name=f"qsT{i}", tag=f"qsT{i}")
                nc.vector.tensor_copy(out=qsT[:], in_=pq[: NL - 1, :])

                # o = qsT.T @ emb_c[1:6]   [128, 768]
                o = sb.tile([P, H], BF16, name=f"o{i}", tag=f"o{i}")
                for c in range(2):
                    po = pso.tile([P, H // 2], F32, name="po", tag="po")
                    nc.tensor.matmul(
                        out=po[:],
                        lhsT=qsT[:],
                        rhs=ec[:, c * (H // 2) : (c + 1) * (H // 2)],
                        start=True,
                        stop=True,
                    )
                    if c == 0:
                        nc.scalar.copy(out=o[:, c * (H // 2) : (c + 1) * (H // 2)],
                                       in_=po[:])
                    else:
                        nc.vector.tensor_copy(
                            out=o[:, c * (H // 2) : (c + 1) * (H // 2)], in_=po[:]
                        )
                nc.sync.dma_start(out=o_d[:, i * H : (i + 1) * H], in_=o[:])

    nc.compile()
    return nc


def _get_nc():
    if "nc" not in _CACHE:
        _CACHE["nc"] = _build()
    return _CACHE["nc"]


def kernel(**inputs):
    ws = np.asarray(inputs["word_seq"]).astype(np.int32)          # [B, W]
    hs = np.asarray(inputs["hidden_state"], dtype=np.float32)
    lvm = np.asarray(inputs["label_value_matrix"]).astype(np.int32)
    ea = np.asarray(inputs["emb_a"], dtype=np.float32)
    lw = np.asarray(inputs["lin_w"], dtype=np.float32)
    lb = np.asarray(inputs["lin_b"], dtype=np.float32)
    ec = np.asarray(inputs["emb_c"], dtype=np.float32)

    nc = _get_nc()

    ea16 = np.ascontiguousarray(ea.astype(BF_NP))
    # lin_w' = [lin_w.T | lin_b]  [768, 301], chunked to [128, 6, 301]
    lwt = np.concatenate([lw.T, lb[:, None]], axis=1).astype(BF_NP)
    lwt = np.ascontiguousarray(
        lwt.reshape(KT, P, EP).transpose(1, 0, 2).reshape(P, KT * EP)
    )
    ec5 = np.ascontiguousarray(ec[1:NL].astype(BF_NP))

    in_maps = []
    for c in range(NCORES):
        b, half = divmod(c, 2)
        lsl = slice(half * LC, (half + 1) * LC)
        hm = hs[b, lsl].T.astype(BF_NP)                   # [768, 256]
        hm = np.ascontiguousarray(
            hm.reshape(KT, P, LC).transpose(1, 0, 2).reshape(P, KT * LC)
        )
        lab = lvm[b, lsl].astype(BF_NP)                   # [256, 256]
        lab = np.ascontiguousarray(
            lab.reshape(LT, P, W).transpose(1, 0, 2).reshape(P, LT * W)
        )
        in_maps.append({
            "emb_a": ea16,
            "widx": np.ascontiguousarray(ws[b].reshape(WT, P).T),
            "hm": hm,
            "lwt": lwt,
            "label": lab,
            "emb_c": ec5,
        })

    res = bass_utils.run_bass_kernel_spmd(
        nc, in_maps, core_ids=list(range(NCORES)), trace=TRACE
    )
    _CACHE["last_result"] = res

    out = np.empty((B, L, H), np.float32)
    for c in range(NCORES):
        b, half = divmod(c, 2)
        oc = res.results[c]["o"].reshape(P, LT, H).transpose(1, 0, 2)
        out[b, half * LC : (half + 1) * LC] = oc.reshape(LC, H).astype(np.float32)
    return out


# revision 21
# speedup vs baseline: 1.6194x; 1.2604x over previous
"""Trainium2 Bass kernel for the GCA sparse-attention module.

Math (per batch b):
    a  = emb_a[word_seq] @ lin_w + lin_b                    # [W, H]
    u  = hidden @ a.T / sqrt(H)                             # [L, W]
    e  = exp(u) * (label > 0)                               # [L, W]
    p  = e / (sum_w e + 1e-10)
    o  = sum_w p * emb_c[label]                             # [L, H]

Restructures vs the straightforward version:
  * labels take only 6 values, so o[l] = (qe[l]/denom[l]) @ emb_c with
    qe[l, n] = sum_w e[l, w] * [label[l, w] == n]  (label 0 dropped: K=5).
  * u is contracted over E, not H:  u = (lin_w' @ h.T).T @ e'.T with the
    bias folded in as a 301st embedding column of ones.
  * everything bf16 on device (1 PE cycle/row vs 4 for fp32; half the DMA),
    fp32 PSUM accumulation; output returned bf16 and upcast on host.
  * gathered-row transpose via the DMA XBAR (no PE transposes / PSUM copies).
  * label one-hot masks precomputed early (Pool + DVE) from the label plane;
    the e-weighted bucket sums use DVE's fused tensor_tensor_reduce.

Sharding: 8 cores = (batch b, L-half) pairs; each core fully independent
(emb_a table replicated; each core gathers its 256 rows via indirect DMA).
"""

import numpy as np
import ml_dtypes

import concourse.bass as bass
import concourse.mybir as mybir
import concourse.tile as tile
from concourse import bacc
from concourse import bass_utils
from concourse.masks import make_identity

# Problem shapes (hardcoded per contract).
B, L, W = 4, 512, 256
VOCAB, E, H = 30000, 300, 768
NL = 6
P = 128
NCORES = 8
LC = L * B // NCORES        # 256 l-rows per core
WT = W // P                 # 2 w-tiles
LT = LC // P                # 2 l-tiles
KT = H // P                 # 6 h-chunks (contraction for hprojT)
EP = E + 1                  # 301: embedding dim + folded bias column
EC = [P, P, EP - 2 * P]     # 301 split into e'-chunks [128, 128, 45]
TEMPER = float(H) ** 0.5

F32 = mybir.dt.float32
BF16 = mybir.dt.bfloat16
I32 = mybir.dt.int32
BF_NP = ml_dtypes.bfloat16

USE_XBAR = True   # DMA XBAR transpose for gathered rows (vs PE transpose)
USE_TTR = False   # fused DVE tensor_tensor_reduce crashes the device (NRT
                  # exec-unit unrecoverable) in this runtime; keep split ops
TRACE = False     # test.py flips this for profiled runs

_CACHE = {}


def _build():
    """Build + compile the per-core Bass program (identical on all cores)."""
    nc = bacc.Bacc("TRN2", debug=False, num_devices=1)

    ea_d = nc.dram_tensor("emb_a", [VOCAB, E], BF16, kind="ExternalInput").ap()
    widx_d = nc.dram_tensor("widx", [P, WT], I32, kind="ExternalInput").ap()
    hm_d = nc.dram_tensor("hm", [P, KT * LC], BF16, kind="ExternalInput").ap()
    lwt_d = nc.dram_tensor("lwt", [P, KT * EP], BF16, kind="ExternalInput").ap()
    lab_d = nc.dram_tensor("label", [P, LT * W], BF16, kind="ExternalInput").ap()
    ec_d = nc.dram_tensor("emb_c", [NL - 1, H], BF16, kind="ExternalInput").ap()
    o_d = nc.dram_tensor("o", [P, LT * H], BF16, kind="ExternalOutput").ap()

    with tile.TileContext(nc) as tc:
        with (
            tc.tile_pool(name="cst", bufs=1) as cst,
            tc.tile_pool(name="sb", bufs=1) as sb,
            tc.tile_pool(name="ps", bufs=3, space="PSUM") as ps,
            tc.tile_pool(name="pst", bufs=2, space="PSUM") as pst,
            tc.tile_pool(name="pso", bufs=2, space="PSUM") as pso,
        ):
            # ---- input DMAs (two dispatch engines so transfers overlap) ----
            wt = cst.tile([P, WT], I32, name="wt")
            nc.sync.dma_start(out=wt[:], in_=widx_d)

            labf = cst.tile([P, LT * W], BF16, name="labf")
            nc.scalar.dma_start(out=labf[:], in_=lab_d)

            hm = cst.tile([P, KT * LC], BF16, name="hm")
            nc.sync.dma_start(out=hm[:], in_=hm_d)

            lwt = cst.tile([P, KT * EP], BF16, name="lwt")
            nc.scalar.dma_start(out=lwt[:], in_=lwt_d)

            ec = cst.tile([NL - 1, H], BF16, name="ec")
            nc.sync.dma_start(out=ec[:], in_=ec_d)

            # ---- gather emb_a rows: aw[j] = emb_a[widx[:, j]]  [128, 300] ----
            # aw is padded to 3*128 cols; col 300 = ones (folded bias), so the
            # transposed chunks carry the ones row at awT2 row 44.
            aw = []
            for j in range(WT):
                t = sb.tile([P, 3 * P], BF16, name=f"aw{j}", tag=f"aw{j}")
                nc.gpsimd.indirect_dma_start(
                    out=t[:, :E],
                    out_offset=None,
                    in_=ea_d,
                    in_offset=bass.IndirectOffsetOnAxis(ap=wt[:, j : j + 1], axis=0),
                )
                nc.gpsimd.memset(t[:, E : E + 1], 1.0)
                nc.gpsimd.memset(t[:, E + 1 :], 0.0)
                aw.append(t)

            # ---- label one-hot masks (early; labels-only dependency) ----
            # All on DVE: ~290ns each in 2x mode ([128, 512], both l-tiles).
            # (GpSimd is ~25x slower for this op — measured 7.9us.)
            masks = {}
            for n in range(1, NL):
                m = sb.tile([P, LT * W], BF16, name=f"m{n}", tag=f"m{n}")
                nc.vector.tensor_scalar(
                    out=m[:], in0=labf[:],
                    scalar1=float(n), scalar2=None,
                    op0=mybir.AluOpType.is_equal,
                )
                masks[n] = m

            # ---- awT[k] = gathered rows transposed  [<=128, 256] ----
            awT = []
            for k in range(3):
                t = sb.tile([P, WT * P], BF16, name=f"awT{k}", tag=f"awT{k}")
                awT.append(t)
            if USE_XBAR:
                # NOTE: dispatch only from the SP queue — XBAR transposes
                # dispatched from the Activation queue crashed the device.
                for j in range(WT):
                    for k in range(3):
                        nc.sync.dma_start_transpose(
                            out=awT[k][:, j * P : (j + 1) * P],
                            in_=aw[j][:, k * P : (k + 1) * P],
                        )
            else:
                ident = cst.tile([P, P], BF16, name="ident")
                make_identity(nc, ident[:])
                cp_eng = [nc.vector, nc.scalar, nc.vector,
                          nc.scalar, nc.vector, nc.scalar]
                for j in range(WT):
                    for k in range(3):
                        pt = pst.tile([P, P], BF16, name="pt", tag="pt")
                        nc.tensor.transpose(
                            out=pt[:], in_=aw[j][:, k * P : (k + 1) * P],
                            identity=ident[:],
                        )
                        e_ = cp_eng[j * 3 + k]
                        if e_ is nc.scalar:
                            e_.copy(out=awT[k][:, j * P : (j + 1) * P], in_=pt[:])
                        else:
                            e_.tensor_copy(out=awT[k][:, j * P : (j + 1) * P],
                                           in_=pt[:])

            # ---- hprojT[t] = (lin_w' @ h.T)[e-tile t]  [ec_t, 256] ----
            # lwt layout: [128, k, 301] (k = h-chunk); hm layout: [128, k, 256]
            hp = []
            for t in range(3):
                ew = EC[t]
                toff = t * P
                pp = ps.tile([P, LC], F32, name="pp", tag="pp")
                for k in range(KT):
                    nc.tensor.matmul(
                        out=pp[:ew, :],
                        lhsT=lwt[:, k * EP + toff : k * EP + toff + ew],
                        rhs=hm[:, k * LC : (k + 1) * LC],
                        start=(k == 0),
                        stop=(k == KT - 1),
                    )
                ht = sb.tile([ew, LC], BF16, name=f"hp{t}", tag=f"hp{t}")
                nc.scalar.copy(out=ht[:], in_=pp[:ew, :])
                hp.append(ht)

            # ---- per l-tile: scores, exp, bucket sums, normalize, output ----
            e_t = []
            for i in range(LT):
                pu = ps.tile([P, W], F32, name="pu", tag="pp")
                for k in range(3):
                    kc = EC[k]
                    nc.tensor.matmul(
                        out=pu[:],
                        lhsT=hp[k][:kc, i * P : (i + 1) * P],
                        rhs=awT[k][:kc, :],
                        start=(k == 0),
                        stop=(k == 2),
                    )
                e = sb.tile([P, W], BF16, name=f"e{i}", tag=f"e{i}")
                nc.scalar.activation(
                    out=e[:], in_=pu[:],
                    func=mybir.ActivationFunctionType.Exp,
                    scale=1.0 / TEMPER,
                )
                e_t.append(e)

            qs_t, r_t = [], []
            for i in range(LT):
                e = e_t[i]
                # products for all 5 labels side by side, then ONE batched
                # reduce [128, 5, 256] -> [128, 5]
                scr = sb.tile([P, (NL - 1) * W], BF16, name=f"scr{i}",
                              tag=f"scr{i}")
                for n in range(1, NL):
                    nc.vector.tensor_tensor(
                        out=scr[:, (n - 1) * W : n * W],
                        in0=masks[n][:, i * W : (i + 1) * W], in1=e[:],
                        op=mybir.AluOpType.mult,
                    )
                qe = sb.tile([P, NL - 1], F32, name=f"qe{i}", tag=f"qe{i}")
                nc.vector.tensor_reduce(
                    out=qe[:], in_=scr[:].rearrange("p (n w) -> p n w", n=NL - 1),
                    axis=mybir.AxisListType.X, op=mybir.AluOpType.add,
                )
                # r = 1/sum_n qe (the 1/denom scale is folded into the
                # output PSUM->SBUF copies); qs = bf16 cast of qe
                s = sb.tile([P, 1], F32, name=f"s{i}", tag=f"s{i}")
                nc.vector.tensor_reduce(
                    out=s[:], in_=qe[:], axis=mybir.AxisListType.X,
                    op=mybir.AluOpType.add,
                )
                r = sb.tile([P, 1], F32, name=f"r{i}", tag=f"r{i}")
                nc.vector.reciprocal(out=r[:], in_=s[:])
                r_t.append(r)
                qs = sb.tile([P, NL - 1], BF16, name=f"qs{i}", tag=f"qs{i}")
                nc.vector.tensor_copy(out=qs[:], in_=qe[:])
                qs_t.append(qs)

            identq = cst.tile([P, P], BF16, name="identq")
            make_identity(nc, identq[:])
            for i in range(LT):
                # qsT = qs.T  [5, 128]
                pq = pst.tile([P, P], BF16, name="pq", tag="pt")
                nc.tensor.transpose(out=pq[: NL - 1, :], in_=qs_t[i][:],
                                    identity=identq[:])
                qsT = sb.tile([NL - 1, P], BF16, name=f"qsT{i}", tag=f"qsT{i}")
                nc.vector.tensor_copy(out=qsT[:], in_=pq[: NL - 1, :])

                # o = (qsT.T @ emb_c[1:6]) * r   [128, 768]
                r = r_t[i]
                o = sb.tile([P, H], BF16, name=f"o{i}", tag=f"o{i}")
                for c in range(2):
                    po = pso.tile([P, H // 2], F32, name="po", tag="po")
                    nc.tensor.matmul(
                        out=po[:],
                        lhsT=qsT[:],
                        rhs=ec[:, c * (H // 2) : (c + 1) * (H // 2)],
                        start=True,
                        stop=True,
                    )
                    osl = o[:, c * (H // 2) : (c + 1) * (H // 2)]
                    if c == 0:
                        nc.scalar.mul(out=osl, in_=po[:], mul=r[:, 0:1])
                    else:
                        nc.vector.tensor_scalar(
                            out=osl, in0=po[:],
                            scalar1=r[:, 0:1], scalar2=None,
                            op0=mybir.AluOpType.mult,
                        )
                nc.sync.dma_start(out=o_d[:, i * H : (i + 1) * H], in_=o[:])

    nc.compile()
    return nc


def _get_nc():
    if "nc" not in _CACHE:
        _CACHE["nc"] = _build()
    return _CACHE["nc"]


def kernel(**inputs):
    ws = np.asarray(inputs["word_seq"]).astype(np.int32)          # [B, W]
    hs = np.asarray(inputs["hidden_state"], dtype=np.float32)
    lvm = np.asarray(inputs["label_value_matrix"]).astype(np.int32)
    ea = np.asarray(inputs["emb_a"], dtype=np.float32)
    lw = np.asarray(inputs["lin_w"], dtype=np.float32)
    lb = np.asarray(inputs["lin_b"], dtype=np.float32)
    ec = np.asarray(inputs["emb_c"], dtype=np.float32)

    nc = _get_nc()

    ea16 = np.ascontiguousarray(ea.astype(BF_NP))
    # lin_w' = [lin_w.T | lin_b]  [768, 301], chunked to [128, 6, 301]
    lwt = np.concatenate([lw.T, lb[:, None]], axis=1).astype(BF_NP)
    lwt = np.ascontiguousarray(
        lwt.reshape(KT, P, EP).transpose(1, 0, 2).reshape(P, KT * EP)
    )
    ec5 = np.ascontiguousarray(ec[1:NL].astype(BF_NP))

    in_maps = []
    for c in range(NCORES):
        b, half = divmod(c, 2)
        lsl = slice(half * LC, (half + 1) * LC)
        hm = hs[b, lsl].T.astype(BF_NP)                   # [768, 256]
        hm = np.ascontiguousarray(
            hm.reshape(KT, P, LC).transpose(1, 0, 2).reshape(P, KT * LC)
        )
        lab = lvm[b, lsl].astype(BF_NP)                   # [256, 256]
        lab = np.ascontiguousarray(
            lab.reshape(LT, P, W).transpose(1, 0, 2).reshape(P, LT * W)
        )
        in_maps.append({
            "emb_a": ea16,
            "widx": np.ascontiguousarray(ws[b].reshape(WT, P).T),
            "hm": hm,
            "lwt": lwt,
            "label": lab,
            "emb_c": ec5,
        })

    res = bass_utils.run_bass_kernel_spmd(
        nc, in_maps, core_ids=list(range(NCORES)), trace=TRACE
    )
    _CACHE["last_result"] = res

    out = np.empty((B, L, H), np.float32)
    for c in range(NCORES):
        b, half = divmod(c, 2)
        oc = res.results[c]["o"].reshape(P, LT, H).transpose(1, 0, 2)
        out[b, half * LC : (half + 1) * LC] = oc.reshape(LC, H).astype(np.float32)
    return out


# revision 28
# speedup vs baseline: 1.8771x; 1.1591x over previous
"""Trainium2 Bass kernel for the GCA sparse-attention module.

Math (per batch b):
    a  = emb_a[word_seq] @ lin_w + lin_b                    # [W, H]
    u  = hidden @ a.T / sqrt(H)                             # [L, W]
    e  = exp(u) * (label > 0)                               # [L, W]
    p  = e / (sum_w e + 1e-10)
    o  = sum_w p * emb_c[label]                             # [L, H]

Restructures vs the straightforward version:
  * labels take only 6 values, so o[l] = (qe[l]/denom[l]) @ emb_c with
    qe[l, n] = sum_w e[l, w] * [label[l, w] == n]  (label 0 dropped: K=5).
  * u is contracted over E, not H:  u = (lin_w' @ h.T).T @ e'.T with the
    bias folded in as a 301st embedding column of ones.
  * everything bf16 on device (1 PE cycle/row vs 4 for fp32; half the DMA),
    fp32 PSUM accumulation; output returned bf16 and upcast on host.
  * gathered-row transpose via the DMA XBAR (no PE transposes / PSUM copies).
  * label one-hot masks precomputed early (Pool + DVE) from the label plane;
    the e-weighted bucket sums use DVE's fused tensor_tensor_reduce.

Sharding: 8 cores = (batch b, L-half) pairs; each core fully independent
(emb_a table replicated; each core gathers its 256 rows via indirect DMA).
"""

import numpy as np
import ml_dtypes

import concourse.bass as bass
import concourse.mybir as mybir
import concourse.tile as tile
from concourse import bacc
from concourse import bass_utils
from concourse.masks import make_identity

# Problem shapes (hardcoded per contract).
B, L, W = 4, 512, 256
VOCAB, E, H = 30000, 300, 768
NL = 6
P = 128
NCORES = 8
LC = L * B // NCORES        # 256 l-rows per core
WT = W // P                 # 2 w-tiles
LT = LC // P                # 2 l-tiles
KT = H // P                 # 6 h-chunks (contraction for hprojT)
EP = E + 1                  # 301: embedding dim + folded bias column
EC = [P, P, EP - 2 * P]     # 301 split into e'-chunks [128, 128, 45]
ED = [P, P, E - 2 * P]      # data-only widths of the gather [128, 128, 44]
TEMPER = float(H) ** 0.5

F32 = mybir.dt.float32
BF16 = mybir.dt.bfloat16
I32 = mybir.dt.int32
BF_NP = ml_dtypes.bfloat16

USE_XBAR = True   # DMA XBAR transpose for gathered rows (vs PE transpose)
USE_TTR = False   # fused DVE tensor_tensor_reduce crashes the device (NRT
                  # exec-unit unrecoverable) in this runtime; keep split ops
TRACE = False     # test.py flips this for profiled runs

_CACHE = {}


def _build():
    """Build + compile the per-core Bass program (identical on all cores)."""
    nc = bacc.Bacc("TRN2", debug=False, num_devices=1)

    KH = KT // 2            # 3 h-chunks per DMA half
    ea_d = nc.dram_tensor("emb_a", [VOCAB, E], BF16, kind="ExternalInput").ap()
    widx_d = nc.dram_tensor("widx", [P, WT], I32, kind="ExternalInput").ap()
    hma_d = nc.dram_tensor("hma", [P, KH * LC], BF16, kind="ExternalInput").ap()
    hmb_d = nc.dram_tensor("hmb", [P, KH * LC], BF16, kind="ExternalInput").ap()
    lwa_d = nc.dram_tensor("lwa", [P, KH * EP], BF16, kind="ExternalInput").ap()
    lwb_d = nc.dram_tensor("lwb", [P, KH * EP], BF16, kind="ExternalInput").ap()
    lab_d = nc.dram_tensor("label", [P, LT * W], BF16, kind="ExternalInput").ap()
    ec_d = nc.dram_tensor("emb_c", [NL - 1, H], BF16, kind="ExternalInput").ap()
    o_d = nc.dram_tensor("o", [P, LT * H], BF16, kind="ExternalOutput").ap()

    with tile.TileContext(nc) as tc:
        with (
            tc.tile_pool(name="cst", bufs=1) as cst,
            tc.tile_pool(name="sb", bufs=1) as sb,
            tc.tile_pool(name="ps", bufs=3, space="PSUM") as ps,
            tc.tile_pool(name="pst", bufs=2, space="PSUM") as pst,
            tc.tile_pool(name="pso", bufs=2, space="PSUM") as pso,
        ):
            # ---- input DMAs: halves split across the two dispatch queues so
            # hprojT can start on the first half while the second streams ----
            wt = cst.tile([P, WT], I32, name="wt")
            nc.sync.dma_start(out=wt[:], in_=widx_d)

            labf = cst.tile([P, LT * W], BF16, name="labf")
            nc.scalar.dma_start(out=labf[:], in_=lab_d)

            hma = cst.tile([P, KH * LC], BF16, name="hma")
            nc.sync.dma_start(out=hma[:], in_=hma_d)
            lwa = cst.tile([P, KH * EP], BF16, name="lwa")
            nc.scalar.dma_start(out=lwa[:], in_=lwa_d)

            lwb = cst.tile([P, KH * EP], BF16, name="lwb")
            nc.sync.dma_start(out=lwb[:], in_=lwb_d)
            hmb = cst.tile([P, KH * LC], BF16, name="hmb")
            nc.scalar.dma_start(out=hmb[:], in_=hmb_d)

            ec = cst.tile([NL - 1, H], BF16, name="ec")
            nc.sync.dma_start(out=ec[:], in_=ec_d)

            # ---- gather emb_a rows: aw[j] = emb_a[widx[:, j]]  [128, 300] ----
            aw = []
            for j in range(WT):
                t = sb.tile([P, E], BF16, name=f"aw{j}", tag=f"aw{j}")
                nc.gpsimd.indirect_dma_start(
                    out=t[:],
                    out_offset=None,
                    in_=ea_d,
                    in_offset=bass.IndirectOffsetOnAxis(ap=wt[:, j : j + 1], axis=0),
                )
                aw.append(t)

            # ---- label one-hot masks (early; labels-only dependency) ----
            # All on DVE: ~290ns each in 2x mode ([128, 512], both l-tiles).
            # (GpSimd is ~25x slower for this op — measured 7.9us.)
            masks = {}
            for n in range(1, NL):
                m = sb.tile([P, LT * W], BF16, name=f"m{n}", tag=f"m{n}")
                nc.vector.tensor_scalar(
                    out=m[:], in0=labf[:],
                    scalar1=float(n), scalar2=None,
                    op0=mybir.AluOpType.is_equal,
                )
                masks[n] = m

            # ---- awT[k] = gathered rows transposed  [ec_k, 256] ----
            # awT2 row 44 = ones (folded bias): memset first, copies fill :44
            ident = cst.tile([P, P], BF16, name="ident")
            make_identity(nc, ident[:])
            awT = []
            for k in range(3):
                t = sb.tile([EC[k], WT * P], BF16, name=f"awT{k}", tag=f"awT{k}")
                awT.append(t)
            nc.gpsimd.memset(awT[2][:, :], 1.0)

            def transpose_j(j):
                for k in range(3):
                    kc = ED[k]
                    pt = pst.tile([P, P], BF16, name="pt", tag="pt")
                    nc.tensor.matmul(
                        out=pt[:kc, :], lhsT=aw[j][:, k * P : k * P + kc],
                        rhs=ident[:], is_transpose=True, skip_group_check=True,
                    )
                    nc.vector.tensor_copy(
                        out=awT[k][:kc, j * P : (j + 1) * P], in_=pt[:kc, :]
                    )

            # ---- hprojT[t] = (lin_w' @ h.T)[e-tile t]  [ec_t, 256] ----
            # k-halves A/B follow the two DMA halves; aw transposes are
            # interleaved into the PE stream where their gathers are ready.
            pp_t = []
            for t in range(3):
                ew = EC[t]
                toff = t * P
                pp = ps.tile([P, LC], F32, name="pp", tag="pp")
                for k in range(KH):
                    nc.tensor.matmul(
                        out=pp[:ew, :],
                        lhsT=lwa[:, k * EP + toff : k * EP + toff + ew],
                        rhs=hma[:, k * LC : (k + 1) * LC],
                        start=(k == 0),
                        stop=False,
                        skip_group_check=True,
                    )
                pp_t.append(pp)

            transpose_j(0)

            hp = []
            for t in range(3):
                ew = EC[t]
                toff = t * P
                pp = pp_t[t]
                for k in range(KH):
                    nc.tensor.matmul(
                        out=pp[:ew, :],
                        lhsT=lwb[:, k * EP + toff : k * EP + toff + ew],
                        rhs=hmb[:, k * LC : (k + 1) * LC],
                        start=False,
                        stop=(k == KH - 1),
                        skip_group_check=True,
                    )
                ht = sb.tile([ew, LC], BF16, name=f"hp{t}", tag=f"hp{t}")
                nc.scalar.copy(out=ht[:], in_=pp[:ew, :])
                hp.append(ht)

            transpose_j(1)

            # ---- per l-tile: scores, exp, bucket sums, normalize, output ----
            e_t = []
            for i in range(LT):
                pu = ps.tile([P, W], F32, name="pu", tag="pp")
                for k in range(3):
                    kc = EC[k]
                    nc.tensor.matmul(
                        out=pu[:],
                        lhsT=hp[k][:kc, i * P : (i + 1) * P],
                        rhs=awT[k][:kc, :],
                        start=(k == 0),
                        stop=(k == 2),
                    )
                e = sb.tile([P, W], BF16, name=f"e{i}", tag=f"e{i}")
                nc.scalar.activation(
                    out=e[:], in_=pu[:],
                    func=mybir.ActivationFunctionType.Exp,
                    scale=1.0 / TEMPER,
                )
                e_t.append(e)

            qs_t, r_t = [], []
            for i in range(LT):
                e = e_t[i]
                # products for all 5 labels side by side, then ONE batched
                # reduce [128, 5, 256] -> [128, 5]
                scr = sb.tile([P, (NL - 1) * W], BF16, name=f"scr{i}",
                              tag=f"scr{i}")
                for n in range(1, NL):
                    nc.vector.tensor_tensor(
                        out=scr[:, (n - 1) * W : n * W],
                        in0=masks[n][:, i * W : (i + 1) * W], in1=e[:],
                        op=mybir.AluOpType.mult,
                    )
                qe = sb.tile([P, NL - 1], F32, name=f"qe{i}", tag=f"qe{i}")
                nc.vector.tensor_reduce(
                    out=qe[:], in_=scr[:].rearrange("p (n w) -> p n w", n=NL - 1),
                    axis=mybir.AxisListType.X, op=mybir.AluOpType.add,
                )
                # r = 1/sum_n qe (the 1/denom scale is folded into the
                # output PSUM->SBUF copies); qs = bf16 cast of qe
                s = sb.tile([P, 1], F32, name=f"s{i}", tag=f"s{i}")
                nc.vector.tensor_reduce(
                    out=s[:], in_=qe[:], axis=mybir.AxisListType.X,
                    op=mybir.AluOpType.add,
                )
                r = sb.tile([P, 1], F32, name=f"r{i}", tag=f"r{i}")
                nc.vector.reciprocal(out=r[:], in_=s[:])
                r_t.append(r)
                qs = sb.tile([P, NL - 1], BF16, name=f"qs{i}", tag=f"qs{i}")
                nc.vector.tensor_copy(out=qs[:], in_=qe[:])
                qs_t.append(qs)

            for i in range(LT):
                # qsT = qs.T  [5, 128]
                pq = pst.tile([P, P], BF16, name="pq", tag="pt")
                nc.tensor.transpose(out=pq[: NL - 1, :], in_=qs_t[i][:],
                                    identity=ident[:])
                qsT = sb.tile([NL - 1, P], BF16, name=f"qsT{i}", tag=f"qsT{i}")
                nc.vector.tensor_copy(out=qsT[:], in_=pq[: NL - 1, :])

                # o = (qsT.T @ emb_c[1:6]) * r   [128, 768]
                r = r_t[i]
                o = sb.tile([P, H], BF16, name=f"o{i}", tag=f"o{i}")
                for c in range(2):
                    po = pso.tile([P, H // 2], F32, name="po", tag="po")
                    nc.tensor.matmul(
                        out=po[:],
                        lhsT=qsT[:],
                        rhs=ec[:, c * (H // 2) : (c + 1) * (H // 2)],
                        start=True,
                        stop=True,
                    )
                    osl = o[:, c * (H // 2) : (c + 1) * (H // 2)]
                    if c == 0:
                        nc.scalar.mul(out=osl, in_=po[:], mul=r[:, 0:1])
                    else:
                        nc.vector.tensor_scalar(
                            out=osl, in0=po[:],
                            scalar1=r[:, 0:1], scalar2=None,
                            op0=mybir.AluOpType.mult,
                        )
                nc.sync.dma_start(out=o_d[:, i * H : (i + 1) * H], in_=o[:])

    nc.compile()
    return nc


def _get_nc():
    if "nc" not in _CACHE:
        _CACHE["nc"] = _build()
    return _CACHE["nc"]


def kernel(**inputs):
    ws = np.asarray(inputs["word_seq"]).astype(np.int32)          # [B, W]
    hs = np.asarray(inputs["hidden_state"], dtype=np.float32)
    lvm = np.asarray(inputs["label_value_matrix"]).astype(np.int32)
    ea = np.asarray(inputs["emb_a"], dtype=np.float32)
    lw = np.asarray(inputs["lin_w"], dtype=np.float32)
    lb = np.asarray(inputs["lin_b"], dtype=np.float32)
    ec = np.asarray(inputs["emb_c"], dtype=np.float32)

    nc = _get_nc()

    KH = KT // 2
    ea16 = np.ascontiguousarray(ea.astype(BF_NP))
    # lin_w' = [lin_w.T | lin_b]  [768, 301], chunked to [128, 6, 301]
    lwt = np.concatenate([lw.T, lb[:, None]], axis=1).astype(BF_NP)
    lwt = lwt.reshape(KT, P, EP).transpose(1, 0, 2)       # [128, 6, 301]
    lwa = np.ascontiguousarray(lwt[:, :KH].reshape(P, KH * EP))
    lwb = np.ascontiguousarray(lwt[:, KH:].reshape(P, KH * EP))
    ec5 = np.ascontiguousarray(ec[1:NL].astype(BF_NP))

    in_maps = []
    for c in range(NCORES):
        b, half = divmod(c, 2)
        lsl = slice(half * LC, (half + 1) * LC)
        hm = hs[b, lsl].T.astype(BF_NP)                   # [768, 256]
        hm = hm.reshape(KT, P, LC).transpose(1, 0, 2)     # [128, 6, 256]
        lab = lvm[b, lsl].astype(BF_NP)                   # [256, 256]
        lab = np.ascontiguousarray(
            lab.reshape(LT, P, W).transpose(1, 0, 2).reshape(P, LT * W)
        )
        in_maps.append({
            "emb_a": ea16,
            "widx": np.ascontiguousarray(ws[b].reshape(WT, P).T),
            "hma": np.ascontiguousarray(hm[:, :KH].reshape(P, KH * LC)),
            "hmb": np.ascontiguousarray(hm[:, KH:].reshape(P, KH * LC)),
            "lwa": lwa,
            "lwb": lwb,
            "label": lab,
            "emb_c": ec5,
        })

    res = bass_utils.run_bass_kernel_spmd(
        nc, in_maps, core_ids=list(range(NCORES)), trace=TRACE
    )
    _CACHE["last_result"] = res

    out = np.empty((B, L, H), np.float32)
    for c in range(NCORES):
        b, half = divmod(c, 2)
        oc = res.results[c]["o"].reshape(P, LT, H).transpose(1, 0, 2)
        out[b, half * LC : (half + 1) * LC] = oc.reshape(LC, H).astype(np.float32)
    return out
